# revision 5
# baseline (speedup 1.0000x reference)
"""GPTQ int4 dequant + matmul kernel for Trainium2, column-parallel over 8 cores.

Computes out = x @ dequant(qweight, qzeros, scales) + bias where
  qweight: [OC//8, IC_total] int32 (nibbles packed along OC rows)
  qzeros:  [G, IC_total//8]  int32 (nibbles packed along IC cols)
  scales:  [G, IC_total]     float32
  x:       [N, OC]           float32
  bias:    [IC_total]        float32
Sharding: IC (out_features) split across 8 cores; x replicated.

v3 design (zero-point-free weight path, host bit-layout prep):
  W_deq[oc, j] = (Wq[oc,j] - zp[g,j]) * s[g,j]
  x @ W_deq    = x @ (Wq * s)  -  xg @ (zp * s)      (rank-G correction)
  where xg[n, g] = sum of x[n, k] over group g (128 contiguous k).

  Host-side layout prep (pure bit/layout transforms, no dequant math):
   - qweight nibbles unpacked to uint8 planes n8[kt*128+rp, j] for k-tile
     kt = rt*8+kk holding original rows oc = (rt*128+rp)*8+kk;
   - x columns permuted to the same k-tile order;
   - scales expanded to sx[rt*128+p, j] = s[rt*8+p//16, j];
   - z2b[g, j] = -zp[g,j]*s[g,j] plus a bias row (rank-33 correction table).

  On-chip weight prep per k-tile: ONE tensor_tensor mult
  (uint8 x fp32 -> bf16, exact integer convert) -> W[kt] [128, ic].

  Main loop per 128-token tile: gpsimd cast-DMA x->bf16, plain transpose
  -> xT [128, 32, 128]; two-stage DVE segmented reduce builds group sums
  xg; PE-transpose + ones row -> corrT [33, 128]; 32 k-tile matmuls over 3
  psum chunks + 1 rank-33 correction matmul per chunk (applies -xg@(zp*s)
  and +bias, closes accumulation); per-chunk ACT drain; DMA out.
"""

import sys

if "/opt/trn_rl_repo" not in sys.path:
    sys.path.insert(0, "/opt/trn_rl_repo")

from contextlib import ExitStack

import numpy as np
import ml_dtypes

from concourse import bacc, bass, mybir, tile

P = 128
PACK = 8

f32 = mybir.dt.float32
bf16 = mybir.dt.bfloat16
i32 = mybir.dt.int32
u8 = mybir.dt.uint8
Alu = mybir.AluOpType

# Full problem dims (hardcoded per harness contract)
N_FULL = 4096
K_FULL = 4096  # OC / in_features (contraction)
IC_TOTAL = 11008
G_FULL = 32
N_CORES = 8
IC_SHARD = IC_TOTAL // N_CORES  # 1376
RT = K_FULL // PACK // P  # 4 packed-row tiles
KT = K_FULL // P  # 32 contraction tiles
NT = N_FULL // P  # 32 token tiles
CHUNKS = [(0, 512), (512, 512), (1024, IC_SHARD - 1024)]


def build(nc, n=N_FULL, k=K_FULL, ic=IC_SHARD, g=G_FULL):
    """Emit the per-core program. All cores run the same program (SPMD)."""
    n8_d = nc.dram_tensor("n8", [k, ic], u8, kind="ExternalInput")
    sx_d = nc.dram_tensor("sx", [k // PACK, ic], f32, kind="ExternalInput")
    z2b_d = nc.dram_tensor("z2b", [g + 1, ic], bf16, kind="ExternalInput")
    x_d = nc.dram_tensor("x", [n, k], f32, kind="ExternalInput")
    id128_d = nc.dram_tensor("id128_f32", [P, P], f32, kind="ExternalInput")
    ones_d = nc.dram_tensor("ones_row", [1, P], bf16, kind="ExternalInput")
    out_d = nc.dram_tensor("out", [n, ic], f32, kind="ExternalOutput")

    with tile.TileContext(nc) as tc, ExitStack() as ctx:
        const = ctx.enter_context(tc.tile_pool(name="const", bufs=1))
        sxpool = ctx.enter_context(tc.tile_pool(name="sx", bufs=1))
        wpool = ctx.enter_context(tc.tile_pool(name="w", bufs=1))
        n8pool = ctx.enter_context(tc.tile_pool(name="n8", bufs=4))
        xbpool = ctx.enter_context(tc.tile_pool(name="xb", bufs=2))
        xtpool = ctx.enter_context(tc.tile_pool(name="xt", bufs=3))
        xgpool = ctx.enter_context(tc.tile_pool(name="xg", bufs=2))
        corrpool = ctx.enter_context(tc.tile_pool(name="corr", bufs=2))
        opool = ctx.enter_context(tc.tile_pool(name="o", bufs=2))
        psum = ctx.enter_context(tc.tile_pool(name="psum", bufs=2, space="PSUM"))
        psum_t = ctx.enter_context(tc.tile_pool(name="psum_t", bufs=2, space="PSUM"))

        # ---- small consts on sync queue (cheap, before x transposes)
        id128 = const.tile([P, P], f32)
        nc.sync.dma_start(out=id128[:], in_=id128_d[:])
        ones = const.tile([1, P], bf16)
        nc.sync.dma_start(out=ones[:], in_=ones_d[:])

        # ---- weight dequant inputs on scalar queue, interleaved so W
        # production stays ahead of nt0's consumption
        sx = [sxpool.tile([P, ic], f32, name=f"sx{rt}") for rt in range(RT)]
        wtiles = [wpool.tile([P, ic], bf16, name=f"W{kt}") for kt in range(KT)]

        def emit_prep(rt):
            nc.scalar.dma_start(out=sx[rt][:], in_=sx_d[rt * P : (rt + 1) * P, :])
            for kk in range(PACK):
                kt = rt * PACK + kk
                n8t = n8pool.tile([P, ic], u8, name="n8t")
                nc.scalar.dma_start(
                    out=n8t[:], in_=n8_d[kt * P : (kt + 1) * P, :]
                )
                nc.vector.tensor_tensor(
                    out=wtiles[kt][:], in0=n8t[:], in1=sx[rt][:], op=Alu.mult
                )

        # ---- per-token-tile x pipeline
        xts = [None] * NT
        corrs = [None] * NT

        def emit_x(nt):
            # x columns are host-permuted: xb[:, kt*128 + rp] = x[:, (rt*128+rp)*8+kk]
            xb = xbpool.tile([P, k], bf16, name="xb")
            nc.gpsimd.dma_start(out=xb[:], in_=x_d[nt * P : (nt + 1) * P, :])
            xt = xtpool.tile([P, KT, P], bf16, name="xT")
            nc.sync.dma_start_transpose(out=xt[:], in_=xb[:])
            xts[nt] = xt
            # group sums over original groups g = rt*8 + q, two-stage:
            xp = xgpool.tile([P, 256], f32, name="xp")
            nc.vector.tensor_reduce(
                out=xp[:],
                in_=xb[:, :].rearrange("p (m i) -> p m i", i=16),
                axis=mybir.AxisListType.X,
                op=Alu.add,
            )
            xg = xgpool.tile([P, g], f32, name="xg")
            nc.vector.tensor_reduce(
                out=xg[:],
                in_=xp[:, :].rearrange("p (rt kk q) -> p rt q kk", rt=RT, kk=PACK),
                axis=mybir.AxisListType.X,
                op=Alu.add,
            )
            pg = psum_t.tile([g, P], f32, name="pst")
            nc.tensor.transpose(pg[:, :], xg[:, :], id128[:])
            corrT = corrpool.tile([g + 1, P], bf16, name="corrT")
            nc.scalar.copy(out=corrT[0:g, :], in_=pg[:, :])
            nc.scalar.copy(out=corrT[g : g + 1, :], in_=ones[:])
            corrs[nt] = corrT

        emit_x(0)
        emit_x(1)
        z2b = const.tile([g + 1, ic], bf16)
        nc.sync.dma_start(out=z2b[:], in_=z2b_d[:])
        for rt in range(RT):
            emit_prep(rt)

        # ---- main loop over token tiles
        for nt in range(NT):
            if nt >= 2:
                emit_x(nt)
            xt = xts[nt]
            ps = psum.tile([P, ic], f32, name="ps")
            for kt in range(KT):
                for c0, cw in CHUNKS:
                    nc.tensor.matmul(
                        ps[:, c0 : c0 + cw],
                        lhsT=xt[:, kt, :],
                        rhs=wtiles[kt][:, c0 : c0 + cw],
                        start=(kt == 0),
                        stop=False,
                    )
            out_sb = opool.tile([P, ic], f32, name="out_sb")
            for c0, cw in CHUNKS:
                # rank-33 correction: adds -xg@(zp*s) and +bias, closes group
                nc.tensor.matmul(
                    ps[:, c0 : c0 + cw],
                    lhsT=corrs[nt][:, :],
                    rhs=z2b[:, c0 : c0 + cw],
                    start=False,
                    stop=True,
                )
                nc.scalar.copy(out=out_sb[:, c0 : c0 + cw], in_=ps[:, c0 : c0 + cw])
                nc.scalar.dma_start(
                    out=out_d[nt * P : (nt + 1) * P, c0 : c0 + cw],
                    in_=out_sb[:, c0 : c0 + cw],
                )
    return nc


def make_const_inputs():
    return {
        "id128_f32": np.eye(P, dtype=np.float32),
        "ones_row": np.ones((1, P), dtype=ml_dtypes.bfloat16),
    }


def make_in_maps(input, qweight, qzeros, scales, bias):
    """Shard + host-side bit/layout prep (nibble planes, x perm, scale expand)."""
    consts = make_const_inputs()
    # permute x columns so k-tile kt=rt*8+kk holds rows oc=(rt*128+rp)*8+kk
    rt = np.arange(RT)[:, None, None]
    kk = np.arange(PACK)[None, :, None]
    rp = np.arange(P)[None, None, :]
    perm = ((rt * P + rp) * PACK + kk).reshape(-1)
    x = np.ascontiguousarray(np.asarray(input, dtype=np.float32)[:, perm])
    # unpack qzeros -> zp [G, IC_TOTAL]
    col = np.arange(IC_TOTAL, dtype=np.int32)
    zp = (qzeros[:, col // PACK] >> ((col % PACK) * 4)[None, :]) & 15
    shifts = (np.arange(PACK, dtype=np.int32) * 4)[None, :, None, None]
    in_maps = []
    for c in range(N_CORES):
        j0, j1 = c * IC_SHARD, (c + 1) * IC_SHARD
        qc = np.asarray(qweight[:, j0:j1])  # [512, ic]
        # n8[(rt*8+kk)*128 + rp, j] = (qc[rt*128+rp, j] >> 4kk) & 15
        n8 = ((qc.reshape(RT, 1, P, IC_SHARD) >> shifts) & 15).astype(np.uint8)
        n8 = np.ascontiguousarray(n8.reshape(K_FULL, IC_SHARD))
        s_c = scales[:, j0:j1].astype(np.float32)  # [G, ic]
        sx = np.repeat(s_c, 16, axis=0)  # [512, ic]: sx[rt*128+p] = s[rt*8+p//16]
        z2b = np.empty((G_FULL + 1, IC_SHARD), dtype=np.float32)
        z2b[:G_FULL] = -(zp[:, j0:j1].astype(np.float32) * s_c)
        z2b[G_FULL] = bias[j0:j1]
        in_maps.append(
            {
                "n8": n8,
                "sx": np.ascontiguousarray(sx),
                "z2b": z2b.astype(ml_dtypes.bfloat16),
                "x": x,
                **consts,
            }
        )
    return in_maps


def kernel(input, qweight, qzeros, scales, bias):
    """Full-problem entry point: shard, run on 8 cores, gather."""
    from concourse.bass_utils import run_bass_kernel_spmd

    nc = bacc.Bacc("TRN2", target_bir_lowering=False, debug=False)
    build(nc)
    nc.compile()

    in_maps = make_in_maps(input, qweight, qzeros, scales, bias)
    res = run_bass_kernel_spmd(nc, in_maps, list(range(N_CORES)))
    outs = [np.asarray(res.results[c]["out"], dtype=np.float32) for c in range(N_CORES)]
    return np.concatenate(outs, axis=1)


# revision 6
# speedup vs baseline: 1.3259x; 1.3259x over previous
"""GPTQ int4 dequant + matmul kernel for Trainium2, column-parallel over 8 cores.

Computes out = x @ dequant(qweight, qzeros, scales) + bias where
  qweight: [OC//8, IC_total] int32 (nibbles packed along OC rows)
  qzeros:  [G, IC_total//8]  int32 (nibbles packed along IC cols)
  scales:  [G, IC_total]     float32
  x:       [N, OC]           float32
  bias:    [IC_total]        float32
Sharding: IC (out_features) split across 8 cores; x replicated.

v3 design (zero-point-free weight path, host bit-layout prep):
  W_deq[oc, j] = (Wq[oc,j] - zp[g,j]) * s[g,j]
  x @ W_deq    = x @ (Wq * s)  -  xg @ (zp * s)      (rank-G correction)
  where xg[n, g] = sum of x[n, k] over group g (128 contiguous k).

  Host-side layout prep (pure bit/layout transforms, no dequant math):
   - qweight nibbles unpacked to uint8 planes n8[kt*128+rp, j] for k-tile
     kt = rt*8+kk holding original rows oc = (rt*128+rp)*8+kk;
   - x columns permuted to the same k-tile order;
   - scales expanded to sx[rt*128+p, j] = s[rt*8+p//16, j];
   - z2b[g, j] = -zp[g,j]*s[g,j] plus a bias row (rank-33 correction table).

  On-chip weight prep per k-tile: ONE tensor_tensor mult
  (uint8 x fp32 -> bf16, exact integer convert) -> W[kt] [128, ic].

  Main loop per 128-token tile: gpsimd cast-DMA x->bf16, plain transpose
  -> xT [128, 32, 128]; two-stage DVE segmented reduce builds group sums
  xg; PE-transpose + ones row -> corrT [33, 128]; 32 k-tile matmuls over 3
  psum chunks + 1 rank-33 correction matmul per chunk (applies -xg@(zp*s)
  and +bias, closes accumulation); per-chunk ACT drain; DMA out.
"""

import sys

if "/opt/trn_rl_repo" not in sys.path:
    sys.path.insert(0, "/opt/trn_rl_repo")

from contextlib import ExitStack

import numpy as np
import ml_dtypes

from concourse import bacc, bass, mybir, tile

P = 128
PACK = 8

f32 = mybir.dt.float32
bf16 = mybir.dt.bfloat16
i32 = mybir.dt.int32
u8 = mybir.dt.uint8
Alu = mybir.AluOpType

# Full problem dims (hardcoded per harness contract)
N_FULL = 4096
K_FULL = 4096  # OC / in_features (contraction)
IC_TOTAL = 11008
G_FULL = 32
N_CORES = 8
IC_SHARD = IC_TOTAL // N_CORES  # 1376
RT = K_FULL // PACK // P  # 4 packed-row tiles
KT = K_FULL // P  # 32 contraction tiles
NT = N_FULL // P  # 32 token tiles
CHUNKS = [(0, 512), (512, 512), (1024, IC_SHARD - 1024)]


def build(nc, n=N_FULL, k=K_FULL, ic=IC_SHARD, g=G_FULL):
    """Emit the per-core program. All cores run the same program (SPMD)."""
    n8_d = nc.dram_tensor("n8", [k, ic], u8, kind="ExternalInput")
    sx_d = nc.dram_tensor("sx", [k // PACK, ic], f32, kind="ExternalInput")
    z2b_d = nc.dram_tensor("z2b", [g + 1, ic], bf16, kind="ExternalInput")
    x_d = nc.dram_tensor("x", [n, k], f32, kind="ExternalInput")
    id128_d = nc.dram_tensor("id128_f32", [P, P], f32, kind="ExternalInput")
    ones_d = nc.dram_tensor("ones_row", [1, P], bf16, kind="ExternalInput")
    out_d = nc.dram_tensor("out", [n, ic], f32, kind="ExternalOutput")

    with tile.TileContext(nc) as tc, ExitStack() as ctx:
        const = ctx.enter_context(tc.tile_pool(name="const", bufs=1))
        sxpool = ctx.enter_context(tc.tile_pool(name="sx", bufs=1))
        wpool = ctx.enter_context(tc.tile_pool(name="w", bufs=1))
        n8pool = ctx.enter_context(tc.tile_pool(name="n8", bufs=4))
        xbpool = ctx.enter_context(tc.tile_pool(name="xb", bufs=2))
        xtpool = ctx.enter_context(tc.tile_pool(name="xt", bufs=3))
        xgpool = ctx.enter_context(tc.tile_pool(name="xg", bufs=2))
        corrpool = ctx.enter_context(tc.tile_pool(name="corr", bufs=2))
        opool = ctx.enter_context(tc.tile_pool(name="o", bufs=2))
        psum = ctx.enter_context(tc.tile_pool(name="psum", bufs=2, space="PSUM"))
        psum_t = ctx.enter_context(tc.tile_pool(name="psum_t", bufs=2, space="PSUM"))

        # ---- small consts on sync queue (cheap, before x transposes)
        id128 = const.tile([P, P], f32)
        nc.sync.dma_start(out=id128[:], in_=id128_d[:])
        ones = const.tile([1, P], bf16)
        nc.sync.dma_start(out=ones[:], in_=ones_d[:])

        # ---- weight dequant inputs on scalar queue, interleaved so W
        # production stays ahead of nt0's consumption
        sx = [sxpool.tile([P, ic], f32, name=f"sx{rt}") for rt in range(RT)]
        wtiles = [wpool.tile([P, ic], bf16, name=f"W{kt}") for kt in range(KT)]

        def emit_prep(rt):
            nc.scalar.dma_start(out=sx[rt][:], in_=sx_d[rt * P : (rt + 1) * P, :])
            for kk in range(PACK):
                kt = rt * PACK + kk
                n8t = n8pool.tile([P, ic], u8, name="n8t")
                nc.scalar.dma_start(
                    out=n8t[:], in_=n8_d[kt * P : (kt + 1) * P, :]
                )
                nc.vector.tensor_tensor(
                    out=wtiles[kt][:], in0=n8t[:], in1=sx[rt][:], op=Alu.mult
                )

        # ---- per-token-tile x pipeline
        xts = [None] * NT
        corrs = [None] * NT

        def emit_x(nt):
            # x columns are host-permuted: xb[:, kt*128 + rp] = x[:, (rt*128+rp)*8+kk]
            xb = xbpool.tile([P, k], bf16, name="xb")
            nc.gpsimd.dma_start(out=xb[:], in_=x_d[nt * P : (nt + 1) * P, :])
            xt = xtpool.tile([P, KT, P], bf16, name="xT")
            nc.sync.dma_start_transpose(out=xt[:], in_=xb[:])
            xts[nt] = xt
            # group sums over original groups g = rt*8 + q, two-stage:
            xp = xgpool.tile([P, 256], f32, name="xp")
            nc.vector.tensor_reduce(
                out=xp[:],
                in_=xb[:, :].rearrange("p (m i) -> p m i", i=16),
                axis=mybir.AxisListType.X,
                op=Alu.add,
            )
            xg = xgpool.tile([P, g], f32, name="xg")
            nc.vector.tensor_reduce(
                out=xg[:],
                in_=xp[:, :].rearrange("p (rt kk q) -> p rt q kk", rt=RT, kk=PACK),
                axis=mybir.AxisListType.X,
                op=Alu.add,
            )
            pg = psum_t.tile([g, P], f32, name="pst")
            nc.tensor.transpose(pg[:, :], xg[:, :], id128[:])
            corrT = corrpool.tile([g + 1, P], bf16, name="corrT")
            nc.scalar.copy(out=corrT[0:g, :], in_=pg[:, :])
            nc.scalar.copy(out=corrT[g : g + 1, :], in_=ones[:])
            corrs[nt] = corrT

        emit_x(0)
        emit_x(1)
        z2b = const.tile([g + 1, ic], bf16)
        nc.sync.dma_start(out=z2b[:], in_=z2b_d[:])
        for rt in range(RT):
            emit_prep(rt)

        # ---- main loop over token tiles
        for nt in range(NT):
            if nt >= 2:
                emit_x(nt)
            xt = xts[nt]
            ps = psum.tile([P, ic], f32, name="ps")
            for kt in range(KT):
                for c0, cw in CHUNKS:
                    nc.tensor.matmul(
                        ps[:, c0 : c0 + cw],
                        lhsT=xt[:, kt, :],
                        rhs=wtiles[kt][:, c0 : c0 + cw],
                        start=(kt == 0),
                        stop=False,
                    )
            # rank-33 correction: adds -xg@(zp*s) and +bias, closes group
            for c0, cw in CHUNKS:
                nc.tensor.matmul(
                    ps[:, c0 : c0 + cw],
                    lhsT=corrs[nt][:, :],
                    rhs=z2b[:, c0 : c0 + cw],
                    start=False,
                    stop=True,
                )
            out_sb = opool.tile([P, ic], f32, name="out_sb")
            nc.scalar.copy(out=out_sb[:], in_=ps[:])
            nc.sync.dma_start(out=out_d[nt * P : (nt + 1) * P, :], in_=out_sb[:])
    return nc


def make_const_inputs():
    return {
        "id128_f32": np.eye(P, dtype=np.float32),
        "ones_row": np.ones((1, P), dtype=ml_dtypes.bfloat16),
    }


def make_in_maps(input, qweight, qzeros, scales, bias):
    """Shard + host-side bit/layout prep (nibble planes, x perm, scale expand)."""
    consts = make_const_inputs()
    # permute x columns so k-tile kt=rt*8+kk holds rows oc=(rt*128+rp)*8+kk
    rt = np.arange(RT)[:, None, None]
    kk = np.arange(PACK)[None, :, None]
    rp = np.arange(P)[None, None, :]
    perm = ((rt * P + rp) * PACK + kk).reshape(-1)
    x = np.ascontiguousarray(np.asarray(input, dtype=np.float32)[:, perm])
    # unpack qzeros -> zp [G, IC_TOTAL]
    col = np.arange(IC_TOTAL, dtype=np.int32)
    zp = (qzeros[:, col // PACK] >> ((col % PACK) * 4)[None, :]) & 15
    shifts = (np.arange(PACK, dtype=np.int32) * 4)[None, :, None, None]
    in_maps = []
    for c in range(N_CORES):
        j0, j1 = c * IC_SHARD, (c + 1) * IC_SHARD
        qc = np.asarray(qweight[:, j0:j1])  # [512, ic]
        # n8[(rt*8+kk)*128 + rp, j] = (qc[rt*128+rp, j] >> 4kk) & 15
        n8 = ((qc.reshape(RT, 1, P, IC_SHARD) >> shifts) & 15).astype(np.uint8)
        n8 = np.ascontiguousarray(n8.reshape(K_FULL, IC_SHARD))
        s_c = scales[:, j0:j1].astype(np.float32)  # [G, ic]
        sx = np.repeat(s_c, 16, axis=0)  # [512, ic]: sx[rt*128+p] = s[rt*8+p//16]
        z2b = np.empty((G_FULL + 1, IC_SHARD), dtype=np.float32)
        z2b[:G_FULL] = -(zp[:, j0:j1].astype(np.float32) * s_c)
        z2b[G_FULL] = bias[j0:j1]
        in_maps.append(
            {
                "n8": n8,
                "sx": np.ascontiguousarray(sx),
                "z2b": z2b.astype(ml_dtypes.bfloat16),
                "x": x,
                **consts,
            }
        )
    return in_maps


def kernel(input, qweight, qzeros, scales, bias):
    """Full-problem entry point: shard, run on 8 cores, gather."""
    from concourse.bass_utils import run_bass_kernel_spmd

    nc = bacc.Bacc("TRN2", target_bir_lowering=False, debug=False)
    build(nc)
    nc.compile()

    in_maps = make_in_maps(input, qweight, qzeros, scales, bias)
    res = run_bass_kernel_spmd(nc, in_maps, list(range(N_CORES)))
    outs = [np.asarray(res.results[c]["out"], dtype=np.float32) for c in range(N_CORES)]
    return np.concatenate(outs, axis=1)
